# revision 12
# baseline (speedup 1.0000x reference)
"""Multi-head graph attention layer (GAT) for Trainium2, 8-core data-parallel.

Problem: B=8, N=1024, D_IN=256, D_OUT=64, H=8, LeakyReLU slope 0.2.
Sharding: one batch element per NeuronCore.

Algebra: with x = f1_i + f2_j and exp monotone, the unnormalized softmax
weight (after factoring out exp(0.2 f1_i), which cancels) is
  U[j,i] = adj[j,i] * max(d_i * E2_j, E2s_j)
with d = exp(0.8 f1), E2 = exp(f2), E2s = exp(0.2 f2).
out^T = [Wh|1]^T @ U gives numerators + the denominator row Z; the
finalize transposes via the DMA XBAR and normalizes.

Engine layout per (pair, jt) with j on partitions, i on free:
  jt0-3: DVE tensor_scalar (mult,max) + DVE quad mask TT over [P,2,2,N]
  jt4-5: ACT relu(E2*d - E2s)  +  DVE scalar_tensor_tensor (+E2s, *adj)
  jt6-7: ACT relu(E2*d - E2s)  +  Pool scalar_tensor_tensor (+E2s, *adj)
All scores/Wh in bf16 (one h load); PE warmup chain ramps the p-state
before the first real matmul; d-broadcast via gpsimd partition_broadcast
(no DRAM bounce); finalize normalizes on Pool and DMAs out per pair.
"""

import numpy as np
import ml_dtypes

BF16 = ml_dtypes.bfloat16

B, N, D_IN, D_OUT, H = 8, 1024, 256, 64, 8
NEG_SLOPE = 0.2
P = 128
NJT = N // P                  # 8 j tiles
NIT = N // P                  # 8 i tiles
NKT = D_IN // P               # 2 contraction tiles
HF = H * D_OUT                # 512
AUG = D_OUT + 1               # 65
TRW = 80                      # transpose row count (65 padded to %16)
NPAIR = H // 2
W12C = 2 * H + 24             # [w2 | zero pad | w1], f1 rows at partition 32


def _build_program():
    import concourse.bass as bass
    import concourse.bacc as bacc
    import concourse.tile as tile
    from concourse import mybir

    f32 = mybir.dt.float32
    bf16 = mybir.dt.bfloat16
    AF = mybir.ActivationFunctionType
    OP = mybir.AluOpType

    nc = bacc.Bacc("TRN2", target_bir_lowering=False, debug=False,
                   enable_asserts=False, num_devices=8)

    hTb = nc.dram_tensor("hTb", [D_IN, N], bf16, kind="ExternalInput").ap()
    adjT = nc.dram_tensor("adjT", [N, N], bf16, kind="ExternalInput").ap()
    wrsb = nc.dram_tensor("wrsb", [D_IN, HF], bf16,
                          kind="ExternalInput").ap()
    w12 = nc.dram_tensor("w12", [D_IN, W12C], bf16,
                         kind="ExternalInput").ap()
    out = nc.dram_tensor("out", [N, HF], bf16, kind="ExternalOutput").ap()

    with tile.TileContext(nc) as tc:
        with (
            tc.tile_pool(name="const", bufs=1) as const,
            tc.tile_pool(name="inputs", bufs=1) as inputs,
            tc.tile_pool(name="whp", bufs=1) as whp,
            tc.tile_pool(name="ecol", bufs=1) as ecolp,
            tc.tile_pool(name="ps_misc", bufs=2, space="PSUM") as ps_misc,
            tc.tile_pool(name="ps_ot", bufs=2, space="PSUM") as ps_ot,
            tc.tile_pool(name="work", bufs=2) as work,
            tc.tile_pool(name="fin", bufs=2) as fin,
            tc.tile_pool(name="dram", bufs=1, space="DRAM") as dramp,
        ):
            # ---- Phase 0: DMA issue (spread across queues) ---------------
            # sync(SP) queue: hTb + w12 (score+Wh critical path)
            htb_sb = inputs.tile([P, NKT, N], bf16)
            w12_sb = inputs.tile([P, NKT, W12C], bf16)
            for kt in range(NKT):
                nc.sync.dma_start(out=htb_sb[:, kt, :],
                                  in_=hTb[kt * P:(kt + 1) * P, :])
                nc.sync.dma_start(out=w12_sb[:, kt, :],
                                  in_=w12[kt * P:(kt + 1) * P, :])
            # scalar(ACT) queue: wrs + first adj tiles
            wrs_sb = inputs.tile([P, NKT, HF], bf16)
            adj_all = inputs.tile([P, NJT, N], bf16)
            for kt in range(NKT):
                nc.scalar.dma_start(out=wrs_sb[:, kt, :],
                                    in_=wrsb[kt * P:(kt + 1) * P, :])
            for jt in range(5):
                nc.scalar.dma_start(out=adj_all[:, jt, :],
                                    in_=adjT[jt * P:(jt + 1) * P, :])
            for jt in range(5, NJT):
                nc.sync.dma_start(out=adj_all[:, jt, :],
                                  in_=adjT[jt * P:(jt + 1) * P, :])

            # Pool: warmup operand, whaug ones columns (before broadcasts)
            z512 = const.tile([P, 512], bf16)
            nc.gpsimd.memset(z512, 0.0)
            whaug = []
            for it in range(NIT):
                wa = whp.tile([P, H, AUG], bf16, tag=f"whaug{it}",
                              name=f"whaug{it}")
                nc.gpsimd.memset(wa[:, :, D_OUT], 1.0)
                whaug.append(wa)

            # ---- PE warmup chain (p-state ramp) --------------------------
            ps_warm = ps_misc.tile([P, 512], f32, tag="m")
            NWARM = 12
            for i in range(NWARM):
                nc.tensor.matmul(ps_warm, z512[:, 0:P], z512,
                                 start=(i == 0), stop=(i == NWARM - 1))

            # ---- Phase 1: scores (bf16) ----------------------------------
            fps = ps_misc.tile([W12C, N], f32, tag="m")
            for half in range(2):
                sl = slice(half * 512, (half + 1) * 512)
                for kt in range(NKT):
                    nc.tensor.matmul(fps[:, sl], w12_sb[:, kt, :],
                                     htb_sb[:, kt, sl],
                                     start=(kt == 0), stop=(kt == NKT - 1))

            # exp rows straight from PSUM
            erows1 = ecolp.tile([2 * H, N], bf16)
            erows2 = ecolp.tile([2 * H, N], bf16)
            dTt = ecolp.tile([W12C, N], bf16)
            nc.scalar.activation(erows1, fps[0:2 * H, :], AF.Exp, scale=1.0)
            nc.scalar.activation(erows2, fps[0:2 * H, :], AF.Exp,
                                 scale=NEG_SLOPE)
            nc.scalar.activation(dTt[32:32 + H, :], fps[32:32 + H, :],
                                 AF.Exp, scale=1.0 - NEG_SLOPE)

            # d broadcast across partitions (DRAM bounce, stride-0 reads)
            dT_dram = dramp.tile([H, N], bf16)
            nc.sync.dma_start(out=dT_dram, in_=dTt[32:32 + H, :])
            dbc_all = ecolp.tile([P, H, N], bf16)
            for h in range(H):
                eng = nc.sync if h % 2 == 0 else nc.scalar
                eng.dma_start(
                    out=dbc_all[:, h, :],
                    in_=dT_dram[h:h + 1, :].partition_broadcast(P))

            # per-i-tile scalar columns via XBAR transpose (issue on SP)
            ec_b1 = ecolp.tile([P, NIT, 2 * H], bf16)
            ec_b2 = ecolp.tile([P, NIT, 2 * H], bf16)
            nc.sync.dma_start_transpose(out=ec_b1, in_=erows1)
            nc.sync.dma_start_transpose(out=ec_b2, in_=erows2)
            # ec_all cols: [0:8]=E2, [8:16]=E2s, [16:24]=-E2s
            ec_all = ecolp.tile([P, NIT, 3 * H], f32)
            nc.vector.tensor_copy(ec_all[:, :, 0:H], ec_b1[:, :, 0:H])
            nc.vector.tensor_copy(ec_all[:, :, H:2 * H], ec_b2[:, :, 0:H])
            nc.vector.tensor_scalar(ec_all[:, :, 2 * H:3 * H],
                                    ec_all[:, :, H:2 * H], -1.0, None,
                                    op0=OP.mult)
            ecols = [ec_all[:, it, :] for it in range(NIT)]

            # ---- Phase 2: Wh (bf16) with aug ones column -----------------
            for it in range(NIT):
                ps2 = ps_misc.tile([P, H, D_OUT], f32, tag="m",
                                   name=f"ps2_{it}")
                for kt in range(NKT):
                    lhsT = htb_sb[:, kt, it * P:(it + 1) * P]
                    nc.tensor.matmul(ps2, lhsT, wrs_sb[:, kt, :],
                                     start=(kt == 0), stop=(kt == NKT - 1))
                nc.scalar.copy(whaug[it][:, :, 0:D_OUT], ps2)

            out_big = whp.tile([P, NIT, HF], bf16)

            # persistent evac targets (garbage rows zeroed once)
            ots_tiles = []
            for i in range(4):
                t = whp.tile([TRW, N], bf16, tag=f"ots{i}", name=f"ots{i}")
                nc.gpsimd.memset(t[D_OUT:TRW, :], 0.0)
                ots_tiles.append(t)

            # ---- Phase 3: per head-pair attention ------------------------
            def _fin_transpose_norm(h0, ots_pair):
                for k in range(2):
                    h = h0 + k
                    ots = ots_pair[k]
                    trs = fin.tile([P, NIT, TRW], bf16, tag="trs",
                                   bufs=2, name="trs")
                    nc.sync.dma_start_transpose(out=trs, in_=ots[0:TRW, :])
                    rc = fin.tile([P, NIT, 1], f32, tag="rc", bufs=2,
                                  name="rc")
                    nc.vector.reciprocal(rc[:, 0:4, :], trs[:, 0:4, 64:65])
                    nc.vector.reciprocal(rc[:, 4:8, :], trs[:, 4:8, 64:65])
                    for g in range(2):
                        src = trs[:, g * 4:(g + 1) * 4, 0:D_OUT]
                        rcb = rc[:, g * 4:(g + 1) * 4, :] \
                            .broadcast_to([P, 4, D_OUT])
                        dst = out_big[:, g * 4:(g + 1) * 4,
                                      h * D_OUT:(h + 1) * D_OUT]
                        nc.gpsimd.tensor_tensor(out=dst, in0=src,
                                                in1=rcb, op=OP.mult)
                # pair output DMA (SP queue)
                csl = slice(h0 * D_OUT, (h0 + 2) * D_OUT)
                try:
                    dst = out.rearrange("(it p) c -> p it c", p=P)[:, :, csl]
                    nc.sync.dma_start(out=dst, in_=out_big[:, :, csl])
                except Exception:
                    for it in range(NIT):
                        eng = nc.sync if it % 2 == 0 else nc.scalar
                        eng.dma_start(out=out[it * P:(it + 1) * P, csl],
                                      in_=out_big[:, it, csl])

            pending = None
            for p in range(NPAIR):
                h0 = 2 * p

                ot = [ps_ot.tile([AUG, N], f32, tag="ot", name=f"ot{k}")
                      for k in range(2)]

                # ACT: evac prev pair's PSUM first, then relus for B lanes
                if pending is not None:
                    ph0, pot, pots = pending
                    for k in range(2):
                        nc.scalar.copy(pots[k][0:AUG, :], pot[k])
                rel = {}
                for jt in (3, 4, 5, 6, 7):
                    r = work.tile([P, 2, N], bf16, tag=f"r{jt}",
                                  name=f"r{jt}")
                    for k in range(2):
                        h = h0 + k
                        nc.scalar.activation(
                            r[:, k, :], dbc_all[:, h, :], AF.Relu,
                            bias=ecols[jt][:, 2 * H + h:2 * H + h + 1],
                            scale=ecols[jt][:, h:h + 1])
                    rel[jt] = r

                # DVE: TS for jt0-2; quad mask (jt0,1); jt2 mask on Pool
                tq = work.tile([P, 2, 2, N], bf16, tag="tq", name="tq")
                for jl in range(2):
                    for k in range(2):
                        h = h0 + k
                        nc.vector.tensor_scalar(
                            tq[:, k, jl, :], dbc_all[:, h, :],
                            ecols[jl][:, h:h + 1],
                            ecols[jl][:, H + h:H + h + 1],
                            op0=OP.mult, op1=OP.max)
                umq = work.tile([P, 2, 2, N], bf16, tag="umq", name="umq")
                adjq = adj_all[:, 0:2, :].unsqueeze(1) \
                    .broadcast_to([P, 2, 2, N])
                nc.vector.tensor_tensor(out=umq, in0=tq, in1=adjq,
                                        op=OP.mult)

                tq2 = work.tile([P, 2, N], bf16, tag="tq2", name="tq2")
                for k in range(2):
                    h = h0 + k
                    nc.vector.tensor_scalar(
                        tq2[:, k, :], dbc_all[:, h, :],
                        ecols[2][:, h:h + 1],
                        ecols[2][:, H + h:H + h + 1],
                        op0=OP.mult, op1=OP.max)
                um2 = work.tile([P, 2, N], bf16, tag="um2", name="um2")
                adj2 = adj_all[:, 2, :].unsqueeze(1) \
                    .broadcast_to([P, 2, N])
                nc.gpsimd.tensor_tensor(out=um2, in0=tq2, in1=adj2,
                                        op=OP.mult)

                # DVE STT for B lanes: umq = (r + E2s) * adj
                umb = {}
                for jt in (3, 4, 5, 6, 7):
                    ub = work.tile([P, 2, N], bf16, tag=f"ub{jt}",
                                   name=f"ub{jt}")
                    for k in range(2):
                        h = h0 + k
                        nc.vector.scalar_tensor_tensor(
                            out=ub[:, k, :], in0=rel[jt][:, k, :],
                            scalar=ecols[jt][:, H + h:H + h + 1],
                            in1=adj_all[:, jt, :],
                            op0=OP.add, op1=OP.mult)
                    umb[jt] = ub

                # Pool: normalize prev pair
                if pending is not None:
                    _fin_transpose_norm(pending[0], pending[2])

                # PE: accumulate in production order jt0,1, 2, 3..7
                def mm(jt, rhs_of_k, start, stop):
                    for k in range(2):
                        lhsT = whaug[jt][:, h0 + k, :]
                        for nh in range(2):
                            nc.tensor.matmul(
                                ot[k][:, nh * 512:(nh + 1) * 512], lhsT,
                                rhs_of_k(k)[:, nh * 512:(nh + 1) * 512],
                                start=start, stop=stop)

                for jl in range(2):
                    mm(jl, lambda k, jl=jl: umq[:, k, jl, :],
                       start=(jl == 0), stop=False)
                mm(2, lambda k: um2[:, k, :], start=False, stop=False)
                for jt in (3, 4, 5, 6):
                    mm(jt, lambda k, jt=jt: umb[jt][:, k, :],
                       start=False, stop=False)
                mm(7, lambda k: umb[7][:, k, :], start=False, stop=True)

                # stage evac targets for next iteration
                ots_pair = [ots_tiles[(2 * p + k) % 4] for k in range(2)]
                pending = (h0, ot, ots_pair)

            # drain: last pair evac + finalize
            ph0, pot, pots = pending
            for k in range(2):
                nc.scalar.copy(pots[k][0:AUG, :], pot[k])
            _fin_transpose_norm(ph0, pots)

    nc.compile()
    return nc


def _host_prep(h, adj, W, a):
    a1, a2 = a[:, :D_OUT], a[:, D_OUT:]
    w1 = np.einsum("hdf,hf->hd", W, a1).astype(np.float32)
    w2 = np.einsum("hdf,hf->hd", W, a2).astype(np.float32)
    w12 = np.concatenate(
        [w2.T, np.zeros((D_IN, 24), np.float32), w1.T], axis=1).astype(BF16)
    wrs = np.ascontiguousarray(
        W.transpose(1, 0, 2).reshape(D_IN, HF)).astype(BF16)
    in_maps = []
    for b in range(B):
        in_maps.append({
            "hTb": np.ascontiguousarray(h[b].T).astype(BF16),
            "adjT": np.ascontiguousarray(adj[b].T).astype(BF16),
            "wrsb": wrs,
            "w12": w12,
        })
    return in_maps


def kernel(h, adj, W, a):
    from concourse.bass_utils import run_bass_kernel_spmd

    in_maps = _host_prep(np.asarray(h), np.asarray(adj),
                         np.asarray(W), np.asarray(a))
    nc = _build_program()
    res = run_bass_kernel_spmd(nc, in_maps, core_ids=list(range(B)))
    out = np.stack([np.asarray(res.results[b]["out"]) for b in range(B)])
    return out.astype(np.float32)


# revision 14
# speedup vs baseline: 1.0880x; 1.0880x over previous
"""Multi-head graph attention layer (GAT) for Trainium2, 8-core data-parallel.

Problem: B=8, N=1024, D_IN=256, D_OUT=64, H=8, LeakyReLU slope 0.2.
Sharding: one batch element per NeuronCore.

Algebra: with x = f1_i + f2_j and exp monotone, the unnormalized softmax
weight (after factoring out exp(0.2 f1_i), which cancels) is
  U[j,i] = adj[j,i] * max(d_i * E2_j, E2s_j)
with d = exp(0.8 f1), E2 = exp(f2), E2s = exp(0.2 f2).
out^T = [Wh|1]^T @ U gives numerators + the denominator row Z; the
finalize transposes via the DMA XBAR and normalizes.

Engine layout per (pair, jt) with j on partitions, i on free:
  jt0-3: DVE tensor_scalar (mult,max) + DVE quad mask TT over [P,2,2,N]
  jt4-5: ACT relu(E2*d - E2s)  +  DVE scalar_tensor_tensor (+E2s, *adj)
  jt6-7: ACT relu(E2*d - E2s)  +  Pool scalar_tensor_tensor (+E2s, *adj)
All scores/Wh in bf16 (one h load); PE warmup chain ramps the p-state
before the first real matmul; d-broadcast via gpsimd partition_broadcast
(no DRAM bounce); finalize normalizes on Pool and DMAs out per pair.
"""

import numpy as np
import ml_dtypes

BF16 = ml_dtypes.bfloat16

B, N, D_IN, D_OUT, H = 8, 1024, 256, 64, 8
NEG_SLOPE = 0.2
P = 128
NJT = N // P                  # 8 j tiles
NIT = N // P                  # 8 i tiles
NKT = D_IN // P               # 2 contraction tiles
HF = H * D_OUT                # 512
AUG = D_OUT + 1               # 65
TRW = 80                      # transpose row count (65 padded to %16)
NPAIR = H // 2
W12C = 2 * H + 24             # [w2 | zero pad | w1], f1 rows at partition 32


def _build_program():
    import concourse.bass as bass
    import concourse.bacc as bacc
    import concourse.tile as tile
    from concourse import mybir

    f32 = mybir.dt.float32
    bf16 = mybir.dt.bfloat16
    AF = mybir.ActivationFunctionType
    OP = mybir.AluOpType

    nc = bacc.Bacc("TRN2", target_bir_lowering=False, debug=False,
                   enable_asserts=False, num_devices=8)

    hTb = nc.dram_tensor("hTb", [D_IN, N], bf16, kind="ExternalInput").ap()
    adjT = nc.dram_tensor("adjT", [N, N], bf16, kind="ExternalInput").ap()
    wrsb = nc.dram_tensor("wrsb", [D_IN, HF], bf16,
                          kind="ExternalInput").ap()
    w12 = nc.dram_tensor("w12", [D_IN, W12C], bf16,
                         kind="ExternalInput").ap()
    out = nc.dram_tensor("out", [N, HF], bf16, kind="ExternalOutput").ap()

    with tile.TileContext(nc) as tc:
        with (
            tc.tile_pool(name="const", bufs=1) as const,
            tc.tile_pool(name="inputs", bufs=1) as inputs,
            tc.tile_pool(name="whp", bufs=1) as whp,
            tc.tile_pool(name="ecol", bufs=1) as ecolp,
            tc.tile_pool(name="ps_misc", bufs=2, space="PSUM") as ps_misc,
            tc.tile_pool(name="ps_ot", bufs=2, space="PSUM") as ps_ot,
            tc.tile_pool(name="work", bufs=1) as work,
            tc.tile_pool(name="ump", bufs=2) as ump,
            tc.tile_pool(name="fin", bufs=2) as fin,
            tc.tile_pool(name="dram", bufs=1, space="DRAM") as dramp,
        ):
            # ---- Phase 0: DMA issue (spread across queues) ---------------
            # sync(SP) queue: hTb + w12 (score+Wh critical path)
            htb_sb = inputs.tile([P, NKT, N], bf16)
            w12_sb = inputs.tile([P, NKT, W12C], bf16)
            for kt in range(NKT):
                nc.sync.dma_start(out=htb_sb[:, kt, :],
                                  in_=hTb[kt * P:(kt + 1) * P, :])
                nc.sync.dma_start(out=w12_sb[:, kt, :],
                                  in_=w12[kt * P:(kt + 1) * P, :])
            # scalar(ACT) queue: wrs + first adj tiles
            wrs_sb = inputs.tile([P, NKT, HF], bf16)
            adj_all = inputs.tile([P, NJT, N], bf16)
            for kt in range(NKT):
                nc.scalar.dma_start(out=wrs_sb[:, kt, :],
                                    in_=wrsb[kt * P:(kt + 1) * P, :])
            for jt in range(5):
                nc.scalar.dma_start(out=adj_all[:, jt, :],
                                    in_=adjT[jt * P:(jt + 1) * P, :])
            for jt in range(5, NJT):
                nc.sync.dma_start(out=adj_all[:, jt, :],
                                  in_=adjT[jt * P:(jt + 1) * P, :])

            # Pool: warmup operand, whaug ones columns (before broadcasts)
            z512 = const.tile([P, 512], bf16)
            nc.gpsimd.memset(z512, 0.0)
            whaug = []
            for it in range(NIT):
                wa = whp.tile([P, H, AUG], bf16, tag=f"whaug{it}",
                              name=f"whaug{it}")
                nc.gpsimd.memset(wa[:, :, D_OUT], 1.0)
                whaug.append(wa)

            # ---- PE warmup chain (p-state ramp) --------------------------
            ps_warm = ps_misc.tile([P, 512], f32, tag="m")
            NWARM = 12
            for i in range(NWARM):
                nc.tensor.matmul(ps_warm, z512[:, 0:P], z512,
                                 start=(i == 0), stop=(i == NWARM - 1))

            # ---- Phase 1: scores (bf16) ----------------------------------
            fps = ps_misc.tile([W12C, N], f32, tag="m")
            for half in range(2):
                sl = slice(half * 512, (half + 1) * 512)
                for kt in range(NKT):
                    nc.tensor.matmul(fps[:, sl], w12_sb[:, kt, :],
                                     htb_sb[:, kt, sl],
                                     start=(kt == 0), stop=(kt == NKT - 1))

            # exp rows straight from PSUM
            erows1 = ecolp.tile([2 * H, N], bf16)
            erows2 = ecolp.tile([2 * H, N], bf16)
            dTt = ecolp.tile([W12C, N], bf16)
            nc.scalar.activation(erows1, fps[0:2 * H, :], AF.Exp, scale=1.0)
            nc.scalar.activation(erows2, fps[0:2 * H, :], AF.Exp,
                                 scale=NEG_SLOPE)
            nc.scalar.activation(dTt[32:32 + H, :], fps[32:32 + H, :],
                                 AF.Exp, scale=1.0 - NEG_SLOPE)

            # d broadcast across partitions (DRAM bounce, stride-0 reads)
            dT_dram = dramp.tile([H, N], bf16)
            nc.sync.dma_start(out=dT_dram, in_=dTt[32:32 + H, :])
            dbc_all = ecolp.tile([P, H, N], bf16)
            for h in range(H):
                eng = nc.sync if h % 2 == 0 else nc.scalar
                eng.dma_start(
                    out=dbc_all[:, h, :],
                    in_=dT_dram[h:h + 1, :].partition_broadcast(P))

            # per-i-tile scalar columns via XBAR transpose (issue on SP)
            ec_b1 = ecolp.tile([P, NIT, 2 * H], bf16)
            ec_b2 = ecolp.tile([P, NIT, 2 * H], bf16)
            nc.sync.dma_start_transpose(out=ec_b1, in_=erows1)
            nc.sync.dma_start_transpose(out=ec_b2, in_=erows2)
            # ec_all cols: [0:8]=E2, [8:16]=E2s, [16:24]=-E2s
            ec_all = ecolp.tile([P, NIT, 3 * H], f32)
            nc.vector.tensor_copy(ec_all[:, :, 0:H], ec_b1[:, :, 0:H])
            nc.vector.tensor_copy(ec_all[:, :, H:2 * H], ec_b2[:, :, 0:H])
            nc.vector.tensor_scalar(ec_all[:, :, 2 * H:3 * H],
                                    ec_all[:, :, H:2 * H], -1.0, None,
                                    op0=OP.mult)
            ecols = [ec_all[:, it, :] for it in range(NIT)]

            # ---- Phase 2: Wh (bf16) with aug ones column -----------------
            for it in range(NIT):
                ps2 = ps_misc.tile([P, H, D_OUT], f32, tag="m",
                                   name=f"ps2_{it}")
                for kt in range(NKT):
                    lhsT = htb_sb[:, kt, it * P:(it + 1) * P]
                    nc.tensor.matmul(ps2, lhsT, wrs_sb[:, kt, :],
                                     start=(kt == 0), stop=(kt == NKT - 1))
                nc.scalar.copy(whaug[it][:, :, 0:D_OUT], ps2)

            out_big = whp.tile([P, NIT, HF], bf16)

            # persistent evac targets (garbage rows zeroed once)
            ots_tiles = []
            for i in range(4):
                t = whp.tile([TRW, N], bf16, tag=f"ots{i}", name=f"ots{i}")
                nc.gpsimd.memset(t[D_OUT:TRW, :], 0.0)
                ots_tiles.append(t)

            # ---- Phase 3: per head-pair attention ------------------------
            def _fin_transpose_norm(h0, ots_pair):
                for k in range(2):
                    h = h0 + k
                    ots = ots_pair[k]
                    trs = fin.tile([P, NIT, TRW], bf16, tag="trs",
                                   bufs=2, name="trs")
                    nc.sync.dma_start_transpose(out=trs, in_=ots[0:TRW, :])
                    rc = fin.tile([P, NIT, 1], f32, tag="rc", bufs=2,
                                  name="rc")
                    nc.vector.reciprocal(rc[:, 0:4, :], trs[:, 0:4, 64:65])
                    nc.vector.reciprocal(rc[:, 4:8, :], trs[:, 4:8, 64:65])
                    for g in range(2):
                        src = trs[:, g * 4:(g + 1) * 4, 0:D_OUT]
                        rcb = rc[:, g * 4:(g + 1) * 4, :] \
                            .broadcast_to([P, 4, D_OUT])
                        dst = out_big[:, g * 4:(g + 1) * 4,
                                      h * D_OUT:(h + 1) * D_OUT]
                        nc.gpsimd.tensor_tensor(out=dst, in0=src,
                                                in1=rcb, op=OP.mult)
                # pair output DMA (SP queue)
                csl = slice(h0 * D_OUT, (h0 + 2) * D_OUT)
                try:
                    dst = out.rearrange("(it p) c -> p it c", p=P)[:, :, csl]
                    nc.sync.dma_start(out=dst, in_=out_big[:, :, csl])
                except Exception:
                    for it in range(NIT):
                        eng = nc.sync if it % 2 == 0 else nc.scalar
                        eng.dma_start(out=out[it * P:(it + 1) * P, csl],
                                      in_=out_big[:, it, csl])

            pending = None
            for p in range(NPAIR):
                h0 = 2 * p

                ot = [ps_ot.tile([AUG, N], f32, tag="ot", name=f"ot{k}")
                      for k in range(2)]

                # DVE: TS for D lanes jt0-3 into the mega-quad tile,
                # one mask TT over [P,2,4,N]
                tqA = work.tile([P, 2, 4, N], bf16, tag="tqA", name="tqA")
                for jt in range(4):
                    for k in range(2):
                        h = h0 + k
                        nc.vector.tensor_scalar(
                            tqA[:, k, jt, :], dbc_all[:, h, :],
                            ecols[jt][:, h:h + 1],
                            ecols[jt][:, H + h:H + h + 1],
                            op0=OP.mult, op1=OP.max)
                umA = ump.tile([P, 2, 4, N], bf16, tag="umA", name="umA")
                adjA = adj_all[:, 0:4, :].unsqueeze(1) \
                    .broadcast_to([P, 2, 4, N])
                nc.vector.tensor_tensor(out=umA, in0=tqA, in1=adjA,
                                        op=OP.mult)

                # jt7: D lane, TS on DVE + mask TT on Pool
                tq7 = work.tile([P, 2, N], bf16, tag="tq7", name="tq7")
                for k in range(2):
                    h = h0 + k
                    nc.vector.tensor_scalar(
                        tq7[:, k, :], dbc_all[:, h, :],
                        ecols[7][:, h:h + 1],
                        ecols[7][:, H + h:H + h + 1],
                        op0=OP.mult, op1=OP.max)
                um7 = ump.tile([P, 2, N], bf16, tag="um7", name="um7")
                adj7 = adj_all[:, 7, :].unsqueeze(1) \
                    .broadcast_to([P, 2, N])
                nc.gpsimd.tensor_tensor(out=um7, in0=tq7, in1=adj7,
                                        op=OP.mult)

                # ACT: evac prev pair's PSUM, then B lanes jt4-6
                # (two ACT passes: r = relu(E2*d - E2s); tq = relu(r + E2s))
                if pending is not None:
                    ph0, pot, pots = pending
                    for k in range(2):
                        nc.scalar.copy(pots[k][0:AUG, :], pot[k])
                tqB = work.tile([P, 2, 2, N], bf16, tag="tqB", name="tqB")
                tq6 = work.tile([P, 2, N], bf16, tag="tq6", name="tq6")
                for jt in (4, 5, 6):
                    r = work.tile([P, 2, N], bf16, tag=f"r{jt}",
                                  name=f"r{jt}")
                    for k in range(2):
                        h = h0 + k
                        nc.scalar.activation(
                            r[:, k, :], dbc_all[:, h, :], AF.Relu,
                            bias=ecols[jt][:, 2 * H + h:2 * H + h + 1],
                            scale=ecols[jt][:, h:h + 1])
                        dst = tq6[:, k, :] if jt == 6 else tqB[:, k, jt - 4, :]
                        nc.scalar.activation(
                            dst, r[:, k, :], AF.Relu,
                            bias=ecols[jt][:, H + h:H + h + 1],
                            scale=1.0)
                umB = ump.tile([P, 2, 2, N], bf16, tag="umB", name="umB")
                adjB = adj_all[:, 4:6, :].unsqueeze(1) \
                    .broadcast_to([P, 2, 2, N])
                nc.vector.tensor_tensor(out=umB, in0=tqB, in1=adjB,
                                        op=OP.mult)
                um6 = ump.tile([P, 2, N], bf16, tag="um6", name="um6")
                adj6 = adj_all[:, 6, :].unsqueeze(1) \
                    .broadcast_to([P, 2, N])
                nc.vector.tensor_tensor(out=um6, in0=tq6, in1=adj6,
                                        op=OP.mult)

                # Pool: normalize prev pair
                if pending is not None:
                    _fin_transpose_norm(pending[0], pending[2])

                # PE: accumulate in production order jt0-3, 7, 4, 5, 6
                def mm(jt, rhs_of_k, start, stop):
                    for k in range(2):
                        lhsT = whaug[jt][:, h0 + k, :]
                        for nh in range(2):
                            nc.tensor.matmul(
                                ot[k][:, nh * 512:(nh + 1) * 512], lhsT,
                                rhs_of_k(k)[:, nh * 512:(nh + 1) * 512],
                                start=start, stop=stop)

                for jt in range(4):
                    mm(jt, lambda k, jt=jt: umA[:, k, jt, :],
                       start=(jt == 0), stop=False)
                mm(7, lambda k: um7[:, k, :], start=False, stop=False)
                for jt in (4, 5):
                    mm(jt, lambda k, jt=jt: umB[:, k, jt - 4, :],
                       start=False, stop=False)
                mm(6, lambda k: um6[:, k, :], start=False, stop=True)

                # stage evac targets for next iteration
                ots_pair = [ots_tiles[(2 * p + k) % 4] for k in range(2)]
                pending = (h0, ot, ots_pair)

            # drain: last pair evac + finalize
            ph0, pot, pots = pending
            for k in range(2):
                nc.scalar.copy(pots[k][0:AUG, :], pot[k])
            _fin_transpose_norm(ph0, pots)

    nc.compile()
    return nc


def _host_prep(h, adj, W, a):
    a1, a2 = a[:, :D_OUT], a[:, D_OUT:]
    w1 = np.einsum("hdf,hf->hd", W, a1).astype(np.float32)
    w2 = np.einsum("hdf,hf->hd", W, a2).astype(np.float32)
    w12 = np.concatenate(
        [w2.T, np.zeros((D_IN, 24), np.float32), w1.T], axis=1).astype(BF16)
    wrs = np.ascontiguousarray(
        W.transpose(1, 0, 2).reshape(D_IN, HF)).astype(BF16)
    in_maps = []
    for b in range(B):
        in_maps.append({
            "hTb": np.ascontiguousarray(h[b].T).astype(BF16),
            "adjT": np.ascontiguousarray(adj[b].T).astype(BF16),
            "wrsb": wrs,
            "w12": w12,
        })
    return in_maps


def kernel(h, adj, W, a):
    from concourse.bass_utils import run_bass_kernel_spmd

    in_maps = _host_prep(np.asarray(h), np.asarray(adj),
                         np.asarray(W), np.asarray(a))
    nc = _build_program()
    res = run_bass_kernel_spmd(nc, in_maps, core_ids=list(range(B)))
    out = np.stack([np.asarray(res.results[b]["out"]) for b in range(B)])
    return out.astype(np.float32)


# revision 16
# speedup vs baseline: 1.2037x; 1.1063x over previous
"""Multi-head graph attention layer (GAT) for Trainium2, 8-core data-parallel.

Problem: B=8, N=1024, D_IN=256, D_OUT=64, H=8, LeakyReLU slope 0.2.
Sharding: one batch element per NeuronCore.

Algebra: with x = f1_i + f2_j and exp monotone, the unnormalized softmax
weight (after factoring out exp(0.2 f1_i), which cancels) is
  U[j,i] = adj[j,i] * max(d_i * E2_j, E2s_j)
with d = exp(0.8 f1), E2 = exp(f2), E2s = exp(0.2 f2).
out^T = [Wh|1]^T @ U gives numerators + the denominator row Z; the
finalize transposes via the DMA XBAR and normalizes.

Measured op costs (ns, effective): DVE TS [P,1024] 457, DVE mask TT
542-568/unit (mega-quad [P,2,4,N]), ACT pass 1040-1230, Pool TT [P,2,N]
4276 (useless for big tiles), Pool small TT ~700.  Hence: all masks on
DVE; B lanes (2 ACT passes) only where DVE saturates; Pool does PSUM
evac + normalize only.  Pair 0 is all-D so ACT pre-computes pair 1's
B tiles during it (deep pipeline fill).
All bf16; PE warmup chain ramps the p-state; dbc/ec DMAs are priority-
ordered ahead of bulk adj loads; per-pair output DMA.
"""

import numpy as np
import ml_dtypes

BF16 = ml_dtypes.bfloat16

B, N, D_IN, D_OUT, H = 8, 1024, 256, 64, 8
NEG_SLOPE = 0.2
P = 128
NJT = N // P                  # 8 j tiles
NIT = N // P                  # 8 i tiles
NKT = D_IN // P               # 2 contraction tiles
HF = H * D_OUT                # 512
AUG = D_OUT + 1               # 65
TRW = 80                      # transpose row count (65 padded to %16)
NPAIR = H // 2
W12C = 2 * H + 24             # [w2 | zero pad | w1], f1 rows at partition 32

# B-lane units per pair (pair 0 all-D): jt4,jt5 both k, jt6 k0
B_UNITS = {1: ((4, 0), (4, 1), (5, 0), (5, 1), (6, 0)),
           2: ((4, 0), (4, 1), (5, 0), (5, 1), (6, 0)),
           3: ((4, 0), (4, 1), (5, 0), (5, 1), (6, 0)),
           0: ()}


def _build_program():
    import concourse.bass as bass
    import concourse.bacc as bacc
    import concourse.tile as tile
    from concourse import mybir

    f32 = mybir.dt.float32
    bf16 = mybir.dt.bfloat16
    AF = mybir.ActivationFunctionType
    OP = mybir.AluOpType

    nc = bacc.Bacc("TRN2", target_bir_lowering=False, debug=False,
                   enable_asserts=False, num_devices=8)

    hTb = nc.dram_tensor("hTb", [D_IN, N], bf16, kind="ExternalInput").ap()
    adjT = nc.dram_tensor("adjT", [N, N], bf16, kind="ExternalInput").ap()
    wrsb = nc.dram_tensor("wrsb", [D_IN, HF], bf16,
                          kind="ExternalInput").ap()
    w12 = nc.dram_tensor("w12", [D_IN, W12C], bf16,
                         kind="ExternalInput").ap()
    out = nc.dram_tensor("out", [N, HF], bf16, kind="ExternalOutput").ap()

    with tile.TileContext(nc) as tc:
        with (
            tc.tile_pool(name="const", bufs=1) as const,
            tc.tile_pool(name="inputs", bufs=1) as inputs,
            tc.tile_pool(name="whp", bufs=1) as whp,
            tc.tile_pool(name="ecol", bufs=1) as ecolp,
            tc.tile_pool(name="ps_f", bufs=1, space="PSUM") as ps_f,
            tc.tile_pool(name="ps_misc", bufs=2, space="PSUM") as ps_misc,
            tc.tile_pool(name="ps_ot", bufs=2, space="PSUM") as ps_ot,
            tc.tile_pool(name="work", bufs=1) as work,
            tc.tile_pool(name="ump", bufs=2) as ump,
            tc.tile_pool(name="fin", bufs=2) as fin,
            tc.tile_pool(name="dram", bufs=1, space="DRAM") as dramp,
        ):
            # ---- Phase 0: DMA issue ----------------------------------------
            # SP: score/Wh inputs first, then adj3/4 while dTt pends
            htb_sb = inputs.tile([P, NKT, N], bf16)
            w12_sb = inputs.tile([P, NKT, W12C], bf16)
            for kt in range(NKT):
                nc.sync.dma_start(out=htb_sb[:, kt, :],
                                  in_=hTb[kt * P:(kt + 1) * P, :])
                nc.sync.dma_start(out=w12_sb[:, kt, :],
                                  in_=w12[kt * P:(kt + 1) * P, :])
            wrs_sb = inputs.tile([P, NKT, HF], bf16)
            adj_all = inputs.tile([P, NJT, N], bf16)
            for kt in range(NKT):
                nc.scalar.dma_start(out=wrs_sb[:, kt, :],
                                    in_=wrsb[kt * P:(kt + 1) * P, :])
            for eng, jt in ((nc.scalar, 0), (nc.scalar, 1), (nc.sync, 3),
                            (nc.sync, 4)):
                eng.dma_start(out=adj_all[:, jt, :],
                              in_=adjT[jt * P:(jt + 1) * P, :])

            # Pool: warmup operand + whaug ones + persistent evac targets
            z512 = const.tile([P, 512], bf16)
            nc.gpsimd.memset(z512, 0.0)
            whaug = []
            for it in range(NIT):
                wa = whp.tile([P, H, AUG], bf16, tag=f"whaug{it}",
                              name=f"whaug{it}")
                nc.gpsimd.memset(wa[:, :, D_OUT], 1.0)
                whaug.append(wa)
            ots_tiles = []
            for i in range(4):
                t = whp.tile([TRW, N], bf16, tag=f"ots{i}", name=f"ots{i}")
                nc.gpsimd.memset(t[D_OUT:TRW, :], 0.0)
                ots_tiles.append(t)

            # ---- PE warmup chain (p-state ramp) ----------------------------
            ps_warm = ps_misc.tile([P, 512], f32, tag="m")
            NWARM = 7
            for i in range(NWARM):
                nc.tensor.matmul(ps_warm, z512[:, 0:P], z512,
                                 start=(i == 0), stop=(i == NWARM - 1))

            # ---- Phase 1: scores (bf16), own PSUM pool ---------------------
            fps = ps_f.tile([W12C, N], f32)
            for half in range(2):
                sl = slice(half * 512, (half + 1) * 512)
                for kt in range(NKT):
                    nc.tensor.matmul(fps[:, sl], w12_sb[:, kt, :],
                                     htb_sb[:, kt, sl],
                                     start=(kt == 0), stop=(kt == NKT - 1))

            # exp rows straight from PSUM; dT first (it gates the dbc chain)
            erows1 = ecolp.tile([2 * H, N], bf16)
            erows2 = ecolp.tile([2 * H, N], bf16)
            dTt = ecolp.tile([W12C, N], bf16)
            nc.scalar.activation(dTt[32:32 + H, :], fps[32:32 + H, :],
                                 AF.Exp, scale=1.0 - NEG_SLOPE)
            nc.scalar.activation(erows1, fps[0:2 * H, :], AF.Exp, scale=1.0)
            nc.scalar.activation(erows2, fps[0:2 * H, :], AF.Exp,
                                 scale=NEG_SLOPE)

            # d broadcast across partitions (DRAM bounce, stride-0 reads)
            dT_dram = dramp.tile([H, N], bf16)
            nc.sync.dma_start(out=dT_dram, in_=dTt[32:32 + H, :])
            dbc_all = ecolp.tile([P, H, N], bf16)
            for h in range(H):
                eng = nc.sync if h % 2 == 0 else nc.scalar
                eng.dma_start(
                    out=dbc_all[:, h, :],
                    in_=dT_dram[h:h + 1, :].partition_broadcast(P))

            # per-i-tile scalar columns via XBAR transpose (issue on SP)
            ec_b1 = ecolp.tile([P, NIT, 2 * H], bf16)
            ec_b2 = ecolp.tile([P, NIT, 2 * H], bf16)
            nc.sync.dma_start_transpose(out=ec_b1, in_=erows1)
            nc.sync.dma_start_transpose(out=ec_b2, in_=erows2)
            # remaining adj tiles (after the latency-critical issues)
            for eng, jt in ((nc.sync, 5), (nc.scalar, 2), (nc.scalar, 6),
                            (nc.scalar, 7)):
                eng.dma_start(out=adj_all[:, jt, :],
                              in_=adjT[jt * P:(jt + 1) * P, :])
            # ec_all cols: [0:8]=E2, [8:16]=E2s, [16:24]=-E2s
            ec_all = ecolp.tile([P, NIT, 3 * H], f32)
            nc.vector.tensor_copy(ec_all[:, :, 0:H], ec_b1[:, :, 0:H])
            nc.vector.tensor_copy(ec_all[:, :, H:2 * H], ec_b2[:, :, 0:H])
            nc.vector.tensor_scalar(ec_all[:, :, 2 * H:3 * H],
                                    ec_all[:, :, H:2 * H], -1.0, None,
                                    op0=OP.mult)
            ecols = [ec_all[:, it, :] for it in range(NIT)]

            # ---- Phase 2: Wh (bf16) with aug ones column -------------------
            for it in range(NIT):
                ps2 = ps_misc.tile([P, H, D_OUT], f32, tag="m",
                                   name=f"ps2_{it}")
                for kt in range(NKT):
                    lhsT = htb_sb[:, kt, it * P:(it + 1) * P]
                    nc.tensor.matmul(ps2, lhsT, wrs_sb[:, kt, :],
                                     start=(kt == 0), stop=(kt == NKT - 1))
                nc.scalar.copy(whaug[it][:, :, 0:D_OUT], ps2)

            out_big = whp.tile([P, NIT, HF], bf16)

            # ---- Phase 3: per head-pair attention --------------------------
            def _fin_transpose_norm(h0, ots_pair):
                for k in range(2):
                    h = h0 + k
                    ots = ots_pair[k]
                    trs = fin.tile([P, NIT, TRW], bf16, tag="trs",
                                   bufs=2, name="trs")
                    nc.sync.dma_start_transpose(out=trs, in_=ots[0:TRW, :])
                    rc = fin.tile([P, NIT, 1], f32, tag="rc", bufs=2,
                                  name="rc")
                    nc.vector.reciprocal(rc[:, 0:4, :], trs[:, 0:4, 64:65])
                    nc.vector.reciprocal(rc[:, 4:8, :], trs[:, 4:8, 64:65])
                    for g in range(2):
                        src = trs[:, g * 4:(g + 1) * 4, 0:D_OUT]
                        rcb = rc[:, g * 4:(g + 1) * 4, :] \
                            .broadcast_to([P, 4, D_OUT])
                        dst = out_big[:, g * 4:(g + 1) * 4,
                                      h * D_OUT:(h + 1) * D_OUT]
                        nc.gpsimd.tensor_tensor(out=dst, in0=src,
                                                in1=rcb, op=OP.mult)
                # pair output DMA (SP queue)
                csl = slice(h0 * D_OUT, (h0 + 2) * D_OUT)
                dst = out.rearrange("(it p) c -> p it c", p=P)[:, :, csl]
                nc.sync.dma_start(out=dst, in_=out_big[:, :, csl])

            def ts_unit(dst, jt, h):
                nc.vector.tensor_scalar(
                    dst, dbc_all[:, h, :],
                    ecols[jt][:, h:h + 1],
                    ecols[jt][:, H + h:H + h + 1],
                    op0=OP.mult, op1=OP.max)

            def relu_unit(r_dst, tq_dst, jt, h):
                nc.scalar.activation(
                    r_dst, dbc_all[:, h, :], AF.Relu,
                    bias=ecols[jt][:, 2 * H + h:2 * H + h + 1],
                    scale=ecols[jt][:, h:h + 1])
                nc.scalar.activation(
                    tq_dst, r_dst, AF.Relu,
                    bias=ecols[jt][:, H + h:H + h + 1],
                    scale=1.0)

            pending = None
            for p in range(NPAIR):
                h0 = 2 * p
                bset = B_UNITS[p]

                ot = [ps_ot.tile([AUG, N], f32, tag="ot", name=f"ot{k}")
                      for k in range(2)]

                # two mega tiles: jt0-3 and jt4-7
                tqA = work.tile([P, 2, 4, N], bf16, tag="tqA", name="tqA")
                tqB = work.tile([P, 2, 4, N], bf16, tag="tqB", name="tqB")

                # ACT: B-lane units (two passes each)
                for (jt, k) in bset:
                    h = h0 + k
                    r = work.tile([P, N], bf16, tag=f"r{jt}{k}",
                                  name=f"r{jt}{k}")
                    relu_unit(r, tqB[:, k, jt - 4, :], jt, h)

                # DVE: TS for all D units, then the two mega mask TTs
                for jt in range(4):
                    for k in range(2):
                        ts_unit(tqA[:, k, jt, :], jt, h0 + k)
                umA = ump.tile([P, 2, 4, N], bf16, tag="umA", name="umA")
                adjA = adj_all[:, 0:4, :].unsqueeze(1) \
                    .broadcast_to([P, 2, 4, N])
                nc.vector.tensor_tensor(out=umA, in0=tqA, in1=adjA,
                                        op=OP.mult)
                for jt in range(4, NJT):
                    for k in range(2):
                        if (jt, k) not in bset:
                            ts_unit(tqB[:, k, jt - 4, :], jt, h0 + k)
                umB = ump.tile([P, 2, 4, N], bf16, tag="umB", name="umB")
                adjB = adj_all[:, 4:8, :].unsqueeze(1) \
                    .broadcast_to([P, 2, 4, N])
                nc.vector.tensor_tensor(out=umB, in0=tqB, in1=adjB,
                                        op=OP.mult)

                # Pool: evac prev pair's PSUM, then normalize prev pair
                if pending is not None:
                    ph0, pot, pots = pending
                    for k in range(2):
                        nc.scalar.copy(pots[k][0:AUG, :], pot[k])
                    _fin_transpose_norm(ph0, pots)

                # PE: accumulate jt0-3 then jt4-7
                def mm(jt, start, stop):
                    src = umA if jt < 4 else umB
                    for k in range(2):
                        lhsT = whaug[jt][:, h0 + k, :]
                        for nh in range(2):
                            nc.tensor.matmul(
                                ot[k][:, nh * 512:(nh + 1) * 512], lhsT,
                                src[:, k, jt % 4, nh * 512:(nh + 1) * 512],
                                start=start, stop=stop)

                for jt in range(NJT):
                    mm(jt, start=(jt == 0), stop=(jt == NJT - 1))

                ots_pair = [ots_tiles[(2 * p + k) % 4] for k in range(2)]
                pending = (h0, ot, ots_pair)

            # drain: last pair evac + finalize
            ph0, pot, pots = pending
            for k in range(2):
                nc.scalar.copy(pots[k][0:AUG, :], pot[k])
            _fin_transpose_norm(ph0, pots)

    nc.compile()
    return nc


def _host_prep(h, adj, W, a):
    a1, a2 = a[:, :D_OUT], a[:, D_OUT:]
    w1 = np.einsum("hdf,hf->hd", W, a1).astype(np.float32)
    w2 = np.einsum("hdf,hf->hd", W, a2).astype(np.float32)
    w12 = np.concatenate(
        [w2.T, np.zeros((D_IN, 24), np.float32), w1.T], axis=1).astype(BF16)
    wrs = np.ascontiguousarray(
        W.transpose(1, 0, 2).reshape(D_IN, HF)).astype(BF16)
    in_maps = []
    for b in range(B):
        in_maps.append({
            "hTb": np.ascontiguousarray(h[b].T).astype(BF16),
            "adjT": np.ascontiguousarray(adj[b].T).astype(BF16),
            "wrsb": wrs,
            "w12": w12,
        })
    return in_maps


def kernel(h, adj, W, a):
    from concourse.bass_utils import run_bass_kernel_spmd

    in_maps = _host_prep(np.asarray(h), np.asarray(adj),
                         np.asarray(W), np.asarray(a))
    nc = _build_program()
    res = run_bass_kernel_spmd(nc, in_maps, core_ids=list(range(B)))
    out = np.stack([np.asarray(res.results[b]["out"]) for b in range(B)])
    return out.astype(np.float32)


# revision 17
# speedup vs baseline: 1.2260x; 1.0186x over previous
"""Multi-head graph attention layer (GAT) for Trainium2, 8-core data-parallel.

Problem: B=8, N=1024, D_IN=256, D_OUT=64, H=8, LeakyReLU slope 0.2.
Sharding: one batch element per NeuronCore.

Algebra: with x = f1_i + f2_j and exp monotone, the unnormalized softmax
weight (after factoring out exp(0.2 f1_i), which cancels) is
  U[j,i] = adj[j,i] * max(d_i * E2_j, E2s_j)
with d = exp(0.8 f1), E2 = exp(f2), E2s = exp(0.2 f2).
out^T = [Wh|1]^T @ U gives numerators + the denominator row Z; the
finalize transposes via the DMA XBAR and normalizes.

Measured op costs (ns, effective): DVE TS [P,1024] 457, DVE mask TT
542-568/unit (mega-quad [P,2,4,N]), ACT pass 1040-1230, Pool TT [P,2,N]
4276 (useless for big tiles), Pool small TT ~700.  Hence: all masks on
DVE; B lanes (2 ACT passes) only where DVE saturates; Pool does PSUM
evac + normalize only.  Pair 0 is all-D so ACT pre-computes pair 1's
B tiles during it (deep pipeline fill).
All bf16; PE warmup chain ramps the p-state; dbc/ec DMAs are priority-
ordered ahead of bulk adj loads; per-pair output DMA.
"""

import numpy as np
import ml_dtypes

BF16 = ml_dtypes.bfloat16

B, N, D_IN, D_OUT, H = 8, 1024, 256, 64, 8
NEG_SLOPE = 0.2
P = 128
NJT = N // P                  # 8 j tiles
NIT = N // P                  # 8 i tiles
NKT = D_IN // P               # 2 contraction tiles
HF = H * D_OUT                # 512
AUG = D_OUT + 1               # 65
TRW = 80                      # transpose row count (65 padded to %16)
NPAIR = H // 2
W12C = 2 * H + 24             # [w2 | zero pad | w1], f1 rows at partition 32

# B-lane units per pair (pair 0 all-D): jt4,jt5 both k, jt6 k0
B_UNITS = {1: ((4, 0), (4, 1), (5, 0), (5, 1), (6, 0)),
           2: ((4, 0), (4, 1), (5, 0), (5, 1), (6, 0)),
           3: ((4, 0), (4, 1), (5, 0), (5, 1), (6, 0)),
           0: ()}


def _build_program():
    import concourse.bass as bass
    import concourse.bacc as bacc
    import concourse.tile as tile
    from concourse import mybir

    f32 = mybir.dt.float32
    bf16 = mybir.dt.bfloat16
    AF = mybir.ActivationFunctionType
    OP = mybir.AluOpType

    nc = bacc.Bacc("TRN2", target_bir_lowering=False, debug=False,
                   enable_asserts=False, num_devices=8)

    hTb = nc.dram_tensor("hTb", [D_IN, N], bf16, kind="ExternalInput").ap()
    adjT = nc.dram_tensor("adjT", [N, N], bf16, kind="ExternalInput").ap()
    wrsb = nc.dram_tensor("wrsb", [D_IN, HF], bf16,
                          kind="ExternalInput").ap()
    w12 = nc.dram_tensor("w12", [D_IN, W12C], bf16,
                         kind="ExternalInput").ap()
    out = nc.dram_tensor("out", [N, HF], bf16, kind="ExternalOutput").ap()

    with tile.TileContext(nc) as tc:
        with (
            tc.tile_pool(name="const", bufs=1) as const,
            tc.tile_pool(name="inputs", bufs=1) as inputs,
            tc.tile_pool(name="whp", bufs=1) as whp,
            tc.tile_pool(name="ecol", bufs=1) as ecolp,
            tc.tile_pool(name="ps_f", bufs=1, space="PSUM") as ps_f,
            tc.tile_pool(name="ps_misc", bufs=2, space="PSUM") as ps_misc,
            tc.tile_pool(name="ps_ot", bufs=2, space="PSUM") as ps_ot,
            tc.tile_pool(name="work", bufs=1) as work,
            tc.tile_pool(name="ump", bufs=2) as ump,
            tc.tile_pool(name="fin", bufs=2) as fin,
            tc.tile_pool(name="dram", bufs=1, space="DRAM") as dramp,
        ):
            # ---- Phase 0: DMA issue ----------------------------------------
            # SP: score/Wh inputs first, then adj3/4 while dTt pends
            htb_sb = inputs.tile([P, NKT, N], bf16)
            w12_sb = inputs.tile([P, NKT, W12C], bf16)
            for kt in range(NKT):
                nc.sync.dma_start(out=htb_sb[:, kt, :],
                                  in_=hTb[kt * P:(kt + 1) * P, :])
                nc.sync.dma_start(out=w12_sb[:, kt, :],
                                  in_=w12[kt * P:(kt + 1) * P, :])
            wrs_sb = inputs.tile([P, NKT, HF], bf16)
            adj_all = inputs.tile([P, NJT, N], bf16)
            for kt in range(NKT):
                nc.scalar.dma_start(out=wrs_sb[:, kt, :],
                                    in_=wrsb[kt * P:(kt + 1) * P, :])
            for eng, jt in ((nc.scalar, 0), (nc.scalar, 1), (nc.sync, 3),
                            (nc.sync, 4)):
                eng.dma_start(out=adj_all[:, jt, :],
                              in_=adjT[jt * P:(jt + 1) * P, :])

            # Pool: warmup operand + whaug ones + persistent evac targets
            z512 = const.tile([P, 512], bf16)
            nc.gpsimd.memset(z512, 0.0)
            whaug = []
            for it in range(NIT):
                wa = whp.tile([P, H, AUG], bf16, tag=f"whaug{it}",
                              name=f"whaug{it}")
                nc.gpsimd.memset(wa[:, :, D_OUT], 1.0)
                whaug.append(wa)
            ots_tiles = []
            for i in range(4):
                t = whp.tile([TRW, N], bf16, tag=f"ots{i}", name=f"ots{i}")
                nc.gpsimd.memset(t[D_OUT:TRW, :], 0.0)
                ots_tiles.append(t)

            # ---- PE warmup chain (p-state ramp) ----------------------------
            ps_warm = ps_misc.tile([P, 512], f32, tag="m")
            NWARM = 7
            for i in range(NWARM):
                nc.tensor.matmul(ps_warm, z512[:, 0:P], z512,
                                 start=(i == 0), stop=(i == NWARM - 1))

            # ---- Phase 1: scores (bf16), own PSUM pool ---------------------
            fps = ps_f.tile([W12C, N], f32)
            for half in range(2):
                sl = slice(half * 512, (half + 1) * 512)
                for kt in range(NKT):
                    nc.tensor.matmul(fps[:, sl], w12_sb[:, kt, :],
                                     htb_sb[:, kt, sl],
                                     start=(kt == 0), stop=(kt == NKT - 1))

            # exp rows straight from PSUM; dT first (it gates the dbc chain)
            erows1 = ecolp.tile([2 * H, N], bf16)
            erows2 = ecolp.tile([2 * H, N], bf16)
            dTt = ecolp.tile([W12C, N], bf16)
            nc.scalar.activation(dTt[32:32 + H, :], fps[32:32 + H, :],
                                 AF.Exp, scale=1.0 - NEG_SLOPE)
            nc.scalar.activation(erows1, fps[0:2 * H, :], AF.Exp, scale=1.0)
            nc.scalar.activation(erows2, fps[0:2 * H, :], AF.Exp,
                                 scale=NEG_SLOPE)

            # d broadcast across partitions (DRAM bounce, stride-0 reads)
            # latency-critical: pair-0 heads + ec transposes ONLY; the rest
            # is deferred so it doesn't clog the DMA engines
            dT_dram = dramp.tile([H, N], bf16)
            nc.sync.dma_start(out=dT_dram, in_=dTt[32:32 + H, :])
            dbc_all = ecolp.tile([P, H, N], bf16)

            def bcast(h, eng):
                eng.dma_start(
                    out=dbc_all[:, h, :],
                    in_=dT_dram[h:h + 1, :].partition_broadcast(P))

            bcast(0, nc.sync)
            bcast(1, nc.scalar)
            ec_b1 = ecolp.tile([P, NIT, 2 * H], bf16)
            ec_b2 = ecolp.tile([P, NIT, 2 * H], bf16)
            nc.sync.dma_start_transpose(out=ec_b1, in_=erows1)
            nc.scalar.dma_start_transpose(out=ec_b2, in_=erows2)
            bcast(2, nc.sync)
            bcast(3, nc.scalar)
            for eng, jt in ((nc.sync, 5), (nc.scalar, 2)):
                eng.dma_start(out=adj_all[:, jt, :],
                              in_=adjT[jt * P:(jt + 1) * P, :])
            bcast(4, nc.sync)
            bcast(5, nc.scalar)
            for eng, jt in ((nc.sync, 6), (nc.scalar, 7)):
                eng.dma_start(out=adj_all[:, jt, :],
                              in_=adjT[jt * P:(jt + 1) * P, :])
            bcast(6, nc.sync)
            bcast(7, nc.scalar)
            # ec_all cols: [0:8]=E2, [8:16]=E2s, [16:24]=-E2s
            ec_all = ecolp.tile([P, NIT, 3 * H], f32)
            nc.vector.tensor_copy(ec_all[:, :, 0:H], ec_b1[:, :, 0:H])
            nc.vector.tensor_copy(ec_all[:, :, H:2 * H], ec_b2[:, :, 0:H])
            nc.vector.tensor_scalar(ec_all[:, :, 2 * H:3 * H],
                                    ec_all[:, :, H:2 * H], -1.0, None,
                                    op0=OP.mult)
            ecols = [ec_all[:, it, :] for it in range(NIT)]

            # ---- Phase 2: Wh (bf16) with aug ones column -------------------
            for it in range(NIT):
                ps2 = ps_misc.tile([P, H, D_OUT], f32, tag="m",
                                   name=f"ps2_{it}")
                for kt in range(NKT):
                    lhsT = htb_sb[:, kt, it * P:(it + 1) * P]
                    nc.tensor.matmul(ps2, lhsT, wrs_sb[:, kt, :],
                                     start=(kt == 0), stop=(kt == NKT - 1))
                nc.scalar.copy(whaug[it][:, :, 0:D_OUT], ps2)

            out_big = whp.tile([P, NIT, HF], bf16)

            # ---- Phase 3: per head-pair attention --------------------------
            def _fin_transpose_norm(h0, ots_pair):
                for k in range(2):
                    h = h0 + k
                    ots = ots_pair[k]
                    trs = fin.tile([P, NIT, TRW], bf16, tag="trs",
                                   bufs=2, name="trs")
                    nc.sync.dma_start_transpose(out=trs, in_=ots[0:TRW, :])
                    rc = fin.tile([P, NIT, 1], f32, tag="rc", bufs=2,
                                  name="rc")
                    nc.vector.reciprocal(rc[:, 0:4, :], trs[:, 0:4, 64:65])
                    nc.vector.reciprocal(rc[:, 4:8, :], trs[:, 4:8, 64:65])
                    for g in range(2):
                        src = trs[:, g * 4:(g + 1) * 4, 0:D_OUT]
                        rcb = rc[:, g * 4:(g + 1) * 4, :] \
                            .broadcast_to([P, 4, D_OUT])
                        dst = out_big[:, g * 4:(g + 1) * 4,
                                      h * D_OUT:(h + 1) * D_OUT]
                        nc.gpsimd.tensor_tensor(out=dst, in0=src,
                                                in1=rcb, op=OP.mult)


            def _out_dma(h0):
                csl = slice(h0 * D_OUT, (h0 + 2) * D_OUT)
                dst = out.rearrange("(it p) c -> p it c", p=P)[:, :, csl]
                nc.sync.dma_start(out=dst, in_=out_big[:, :, csl])

            def ts_unit(dst, jt, h):
                nc.vector.tensor_scalar(
                    dst, dbc_all[:, h, :],
                    ecols[jt][:, h:h + 1],
                    ecols[jt][:, H + h:H + h + 1],
                    op0=OP.mult, op1=OP.max)

            def relu_unit(r_dst, tq_dst, jt, h):
                nc.scalar.activation(
                    r_dst, dbc_all[:, h, :], AF.Relu,
                    bias=ecols[jt][:, 2 * H + h:2 * H + h + 1],
                    scale=ecols[jt][:, h:h + 1])
                nc.scalar.activation(
                    tq_dst, r_dst, AF.Relu,
                    bias=ecols[jt][:, H + h:H + h + 1],
                    scale=1.0)

            pending = None
            for p in range(NPAIR):
                h0 = 2 * p
                bset = B_UNITS[p]

                ot = [ps_ot.tile([AUG, N], f32, tag="ot", name=f"ot{k}")
                      for k in range(2)]

                # two mega tiles: jt0-3 and jt4-7
                tqA = work.tile([P, 2, 4, N], bf16, tag="tqA", name="tqA")
                tqB = work.tile([P, 2, 4, N], bf16, tag="tqB", name="tqB")

                # ACT: B-lane units (two passes each)
                for (jt, k) in bset:
                    h = h0 + k
                    r = work.tile([P, N], bf16, tag=f"r{jt}{k}",
                                  name=f"r{jt}{k}")
                    relu_unit(r, tqB[:, k, jt - 4, :], jt, h)

                # DVE: TS for all D units, then the two mega mask TTs
                for jt in range(4):
                    for k in range(2):
                        ts_unit(tqA[:, k, jt, :], jt, h0 + k)
                umA = ump.tile([P, 2, 4, N], bf16, tag="umA", name="umA")
                adjA = adj_all[:, 0:4, :].unsqueeze(1) \
                    .broadcast_to([P, 2, 4, N])
                nc.vector.tensor_tensor(out=umA, in0=tqA, in1=adjA,
                                        op=OP.mult)
                for jt in range(4, NJT):
                    for k in range(2):
                        if (jt, k) not in bset:
                            ts_unit(tqB[:, k, jt - 4, :], jt, h0 + k)
                umB = ump.tile([P, 2, 4, N], bf16, tag="umB", name="umB")
                adjB = adj_all[:, 4:8, :].unsqueeze(1) \
                    .broadcast_to([P, 2, 4, N])
                nc.vector.tensor_tensor(out=umB, in0=tqB, in1=adjB,
                                        op=OP.mult)

                # Pool: evac prev pair's PSUM, then normalize prev pair;
                # out DMA deferred one more pair to keep SP free for
                # the latency-critical transposes
                if pending is not None:
                    ph0, pot, pots = pending
                    for k in range(2):
                        nc.scalar.copy(pots[k][0:AUG, :], pot[k])
                    _fin_transpose_norm(ph0, pots)
                if p >= 2:
                    _out_dma(2 * (p - 2))

                # PE: accumulate jt0-3 then jt4-7
                def mm(jt, start, stop):
                    src = umA if jt < 4 else umB
                    for k in range(2):
                        lhsT = whaug[jt][:, h0 + k, :]
                        for nh in range(2):
                            nc.tensor.matmul(
                                ot[k][:, nh * 512:(nh + 1) * 512], lhsT,
                                src[:, k, jt % 4, nh * 512:(nh + 1) * 512],
                                start=start, stop=stop)

                for jt in range(NJT):
                    mm(jt, start=(jt == 0), stop=(jt == NJT - 1))

                ots_pair = [ots_tiles[(2 * p + k) % 4] for k in range(2)]
                pending = (h0, ot, ots_pair)

            # drain: last pair evac + finalize + remaining out DMAs
            ph0, pot, pots = pending
            for k in range(2):
                nc.scalar.copy(pots[k][0:AUG, :], pot[k])
            _fin_transpose_norm(ph0, pots)
            _out_dma(2)
            _out_dma(4)
            _out_dma(6)

    nc.compile()
    return nc


def _host_prep(h, adj, W, a):
    a1, a2 = a[:, :D_OUT], a[:, D_OUT:]
    w1 = np.einsum("hdf,hf->hd", W, a1).astype(np.float32)
    w2 = np.einsum("hdf,hf->hd", W, a2).astype(np.float32)
    w12 = np.concatenate(
        [w2.T, np.zeros((D_IN, 24), np.float32), w1.T], axis=1).astype(BF16)
    wrs = np.ascontiguousarray(
        W.transpose(1, 0, 2).reshape(D_IN, HF)).astype(BF16)
    in_maps = []
    for b in range(B):
        in_maps.append({
            "hTb": np.ascontiguousarray(h[b].T).astype(BF16),
            "adjT": np.ascontiguousarray(adj[b].T).astype(BF16),
            "wrsb": wrs,
            "w12": w12,
        })
    return in_maps


def kernel(h, adj, W, a):
    from concourse.bass_utils import run_bass_kernel_spmd

    in_maps = _host_prep(np.asarray(h), np.asarray(adj),
                         np.asarray(W), np.asarray(a))
    nc = _build_program()
    res = run_bass_kernel_spmd(nc, in_maps, core_ids=list(range(B)))
    out = np.stack([np.asarray(res.results[b]["out"]) for b in range(B)])
    return out.astype(np.float32)


# revision 18
# speedup vs baseline: 1.2289x; 1.0024x over previous
"""Multi-head graph attention layer (GAT) for Trainium2, 8-core data-parallel.

Problem: B=8, N=1024, D_IN=256, D_OUT=64, H=8, LeakyReLU slope 0.2.
Sharding: one batch element per NeuronCore.

Algebra: with x = f1_i + f2_j and exp monotone, the unnormalized softmax
weight (after factoring out exp(0.2 f1_i), which cancels) is
  U[j,i] = adj[j,i] * max(d_i * E2_j, E2s_j)
with d = exp(0.8 f1), E2 = exp(f2), E2s = exp(0.2 f2).
out^T = [Wh|1]^T @ U gives numerators + the denominator row Z; the
finalize transposes via the DMA XBAR and normalizes.

Measured op costs (ns, effective): DVE TS [P,1024] 457, DVE mask TT
542-568/unit (mega-quad [P,2,4,N]), ACT pass 1040-1230, Pool TT [P,2,N]
4276 (useless for big tiles), Pool small TT ~700.  Hence: all masks on
DVE; B lanes (2 ACT passes) only where DVE saturates; Pool does PSUM
evac + normalize only.  Pair 0 is all-D so ACT pre-computes pair 1's
B tiles during it (deep pipeline fill).
All bf16; PE warmup chain ramps the p-state; dbc/ec DMAs are priority-
ordered ahead of bulk adj loads; per-pair output DMA.
"""

import numpy as np
import ml_dtypes

BF16 = ml_dtypes.bfloat16

B, N, D_IN, D_OUT, H = 8, 1024, 256, 64, 8
NEG_SLOPE = 0.2
P = 128
NJT = N // P                  # 8 j tiles
NIT = N // P                  # 8 i tiles
NKT = D_IN // P               # 2 contraction tiles
HF = H * D_OUT                # 512
AUG = D_OUT + 1               # 65
TRW = 80                      # transpose row count (65 padded to %16)
NPAIR = H // 2
W12C = 2 * H + 24             # [w2 | zero pad | w1], f1 rows at partition 32

# B-lane units per pair (pair 0 all-D): jt4,jt5 both k, jt6 k0
B_UNITS = {1: ((4, 0), (4, 1), (5, 0), (5, 1), (6, 0)),
           2: ((4, 0), (4, 1), (5, 0), (5, 1), (6, 0)),
           3: ((4, 0), (4, 1), (5, 0), (5, 1), (6, 0)),
           0: ()}


def _build_program():
    import concourse.bass as bass
    import concourse.bacc as bacc
    import concourse.tile as tile
    from concourse import mybir

    f32 = mybir.dt.float32
    bf16 = mybir.dt.bfloat16
    AF = mybir.ActivationFunctionType
    OP = mybir.AluOpType

    nc = bacc.Bacc("TRN2", target_bir_lowering=False, debug=False,
                   enable_asserts=False, num_devices=8)

    hTb = nc.dram_tensor("hTb", [D_IN, N], bf16, kind="ExternalInput").ap()
    adjT = nc.dram_tensor("adjT", [N, N], bf16, kind="ExternalInput").ap()
    wrsb = nc.dram_tensor("wrsb", [D_IN, HF], bf16,
                          kind="ExternalInput").ap()
    w12 = nc.dram_tensor("w12", [D_IN, W12C], bf16,
                         kind="ExternalInput").ap()
    out = nc.dram_tensor("out", [N, HF], bf16, kind="ExternalOutput").ap()

    with tile.TileContext(nc) as tc:
        with (
            tc.tile_pool(name="const", bufs=1) as const,
            tc.tile_pool(name="inputs", bufs=1) as inputs,
            tc.tile_pool(name="whp", bufs=1) as whp,
            tc.tile_pool(name="ecol", bufs=1) as ecolp,
            tc.tile_pool(name="ps_f", bufs=1, space="PSUM") as ps_f,
            tc.tile_pool(name="ps_misc", bufs=2, space="PSUM") as ps_misc,
            tc.tile_pool(name="ps_ot", bufs=2, space="PSUM") as ps_ot,
            tc.tile_pool(name="work", bufs=1) as work,
            tc.tile_pool(name="ump", bufs=2) as ump,
            tc.tile_pool(name="fin", bufs=2) as fin,
            tc.tile_pool(name="dram", bufs=1, space="DRAM") as dramp,
        ):
            # ---- Phase 0: DMA issue ----------------------------------------
            # SP: score/Wh inputs first, then adj3/4 while dTt pends
            htb_sb = inputs.tile([P, NKT, N], bf16)
            w12_sb = inputs.tile([P, NKT, W12C], bf16)
            for kt in range(NKT):
                nc.sync.dma_start(out=htb_sb[:, kt, :],
                                  in_=hTb[kt * P:(kt + 1) * P, :])
                nc.sync.dma_start(out=w12_sb[:, kt, :],
                                  in_=w12[kt * P:(kt + 1) * P, :])
            wrs_sb = inputs.tile([P, NKT, HF], bf16)
            adj_all = inputs.tile([P, NJT, N], bf16)
            for kt in range(NKT):
                nc.scalar.dma_start(out=wrs_sb[:, kt, :],
                                    in_=wrsb[kt * P:(kt + 1) * P, :])

            # Pool: warmup operand + whaug ones + persistent evac targets
            z512 = const.tile([P, 512], bf16)
            nc.gpsimd.memset(z512, 0.0)
            whaug = []
            for it in range(NIT):
                wa = whp.tile([P, H, AUG], bf16, tag=f"whaug{it}",
                              name=f"whaug{it}")
                nc.gpsimd.memset(wa[:, :, D_OUT], 1.0)
                whaug.append(wa)
            ots_tiles = []
            for i in range(4):
                t = whp.tile([TRW, N], bf16, tag=f"ots{i}", name=f"ots{i}")
                nc.gpsimd.memset(t[D_OUT:TRW, :], 0.0)
                ots_tiles.append(t)

            # ---- PE warmup chain (p-state ramp) ----------------------------
            ps_warm = ps_misc.tile([P, 512], f32, tag="m")
            NWARM = 7
            for i in range(NWARM):
                nc.tensor.matmul(ps_warm, z512[:, 0:P], z512,
                                 start=(i == 0), stop=(i == NWARM - 1))

            # ---- Phase 1: scores (bf16), own PSUM pool ---------------------
            fps = ps_f.tile([W12C, N], f32)
            for half in range(2):
                sl = slice(half * 512, (half + 1) * 512)
                for kt in range(NKT):
                    nc.tensor.matmul(fps[:, sl], w12_sb[:, kt, :],
                                     htb_sb[:, kt, sl],
                                     start=(kt == 0), stop=(kt == NKT - 1))

            # exp rows straight from PSUM; dT first (it gates the dbc
            # chain).  The whole exp -> dT bounce -> dbc h0/h1 -> ec
            # transpose -> ec assembly chain is the latency-critical path
            # to the first attention op: pin it at high priority so the
            # Tile scheduler doesn't defer any link of it.
            erows1 = ecolp.tile([2 * H, N], bf16)
            erows2 = ecolp.tile([2 * H, N], bf16)
            dTt = ecolp.tile([W12C, N], bf16)
            dT_dram = dramp.tile([H, N], bf16)
            dbc_all = ecolp.tile([P, H, N], bf16)
            ec_b1 = ecolp.tile([P, NIT, 2 * H], bf16)
            ec_b2 = ecolp.tile([P, NIT, 2 * H], bf16)
            ec_all = ecolp.tile([P, NIT, 3 * H], f32)

            def bcast(h, eng):
                eng.dma_start(
                    out=dbc_all[:, h, :],
                    in_=dT_dram[h:h + 1, :].partition_broadcast(P))

            with tc.high_priority():
                nc.scalar.activation(dTt[32:32 + H, :], fps[32:32 + H, :],
                                     AF.Exp, scale=1.0 - NEG_SLOPE)
                nc.scalar.activation(erows1, fps[0:2 * H, :], AF.Exp,
                                     scale=1.0)
                nc.scalar.activation(erows2, fps[0:2 * H, :], AF.Exp,
                                     scale=NEG_SLOPE)
                nc.sync.dma_start(out=dT_dram, in_=dTt[32:32 + H, :])
                bcast(0, nc.sync)
                nc.sync.dma_start_transpose(out=ec_b1, in_=erows1)
                nc.sync.dma_start_transpose(out=ec_b2, in_=erows2)
                bcast(1, nc.sync)
                # ec_all cols: [0:8]=E2, [8:16]=E2s, [16:24]=-E2s
                nc.vector.tensor_copy(ec_all[:, :, 0:H], ec_b1[:, :, 0:H])
                nc.vector.tensor_copy(ec_all[:, :, H:2 * H],
                                      ec_b2[:, :, 0:H])
                nc.vector.tensor_scalar(ec_all[:, :, 2 * H:3 * H],
                                        ec_all[:, :, H:2 * H], -1.0, None,
                                        op0=OP.mult)
            ecols = [ec_all[:, it, :] for it in range(NIT)]

            # deferred: remaining broadcasts + adj tiles (DMA bandwidth)
            for eng, jt in ((nc.scalar, 0), (nc.scalar, 1), (nc.sync, 2),
                            (nc.sync, 3)):
                eng.dma_start(out=adj_all[:, jt, :],
                              in_=adjT[jt * P:(jt + 1) * P, :])
            bcast(2, nc.sync)
            bcast(3, nc.scalar)
            for eng, jt in ((nc.sync, 4), (nc.scalar, 5)):
                eng.dma_start(out=adj_all[:, jt, :],
                              in_=adjT[jt * P:(jt + 1) * P, :])
            bcast(4, nc.sync)
            bcast(5, nc.scalar)
            for eng, jt in ((nc.sync, 6), (nc.scalar, 7)):
                eng.dma_start(out=adj_all[:, jt, :],
                              in_=adjT[jt * P:(jt + 1) * P, :])
            bcast(6, nc.sync)
            bcast(7, nc.scalar)

            # ---- Phase 2: Wh (bf16) with aug ones column -------------------
            for it in range(NIT):
                ps2 = ps_misc.tile([P, H, D_OUT], f32, tag="m",
                                   name=f"ps2_{it}")
                for kt in range(NKT):
                    lhsT = htb_sb[:, kt, it * P:(it + 1) * P]
                    nc.tensor.matmul(ps2, lhsT, wrs_sb[:, kt, :],
                                     start=(kt == 0), stop=(kt == NKT - 1))
                nc.scalar.copy(whaug[it][:, :, 0:D_OUT], ps2)

            out_big = whp.tile([P, NIT, HF], bf16)

            # ---- Phase 3: per head-pair attention --------------------------
            def _fin_transpose_norm(h0, ots_pair):
                for k in range(2):
                    h = h0 + k
                    ots = ots_pair[k]
                    trs = fin.tile([P, NIT, TRW], bf16, tag="trs",
                                   bufs=2, name="trs")
                    nc.sync.dma_start_transpose(out=trs, in_=ots[0:TRW, :])
                    rc = fin.tile([P, NIT, 1], f32, tag="rc", bufs=2,
                                  name="rc")
                    nc.vector.reciprocal(rc[:, 0:4, :], trs[:, 0:4, 64:65])
                    nc.vector.reciprocal(rc[:, 4:8, :], trs[:, 4:8, 64:65])
                    for g in range(2):
                        src = trs[:, g * 4:(g + 1) * 4, 0:D_OUT]
                        rcb = rc[:, g * 4:(g + 1) * 4, :] \
                            .broadcast_to([P, 4, D_OUT])
                        dst = out_big[:, g * 4:(g + 1) * 4,
                                      h * D_OUT:(h + 1) * D_OUT]
                        nc.gpsimd.tensor_tensor(out=dst, in0=src,
                                                in1=rcb, op=OP.mult)


            def _out_dma(h0):
                csl = slice(h0 * D_OUT, (h0 + 2) * D_OUT)
                dst = out.rearrange("(it p) c -> p it c", p=P)[:, :, csl]
                nc.sync.dma_start(out=dst, in_=out_big[:, :, csl])

            def ts_unit(dst, jt, h):
                nc.vector.tensor_scalar(
                    dst, dbc_all[:, h, :],
                    ecols[jt][:, h:h + 1],
                    ecols[jt][:, H + h:H + h + 1],
                    op0=OP.mult, op1=OP.max)

            def relu_unit(r_dst, tq_dst, jt, h):
                nc.scalar.activation(
                    r_dst, dbc_all[:, h, :], AF.Relu,
                    bias=ecols[jt][:, 2 * H + h:2 * H + h + 1],
                    scale=ecols[jt][:, h:h + 1])
                nc.scalar.activation(
                    tq_dst, r_dst, AF.Relu,
                    bias=ecols[jt][:, H + h:H + h + 1],
                    scale=1.0)

            pending = None
            for p in range(NPAIR):
                h0 = 2 * p
                bset = B_UNITS[p]

                ot = [ps_ot.tile([AUG, N], f32, tag="ot", name=f"ot{k}")
                      for k in range(2)]

                # two mega tiles: jt0-3 and jt4-7
                tqA = work.tile([P, 2, 4, N], bf16, tag="tqA", name="tqA")
                tqB = work.tile([P, 2, 4, N], bf16, tag="tqB", name="tqB")

                # ACT: B-lane units (two passes each)
                for (jt, k) in bset:
                    h = h0 + k
                    r = work.tile([P, N], bf16, tag=f"r{jt}{k}",
                                  name=f"r{jt}{k}")
                    relu_unit(r, tqB[:, k, jt - 4, :], jt, h)

                # DVE: TS for all D units, then the two mega mask TTs
                for jt in range(4):
                    for k in range(2):
                        ts_unit(tqA[:, k, jt, :], jt, h0 + k)
                umA = ump.tile([P, 2, 4, N], bf16, tag="umA", name="umA")
                adjA = adj_all[:, 0:4, :].unsqueeze(1) \
                    .broadcast_to([P, 2, 4, N])
                nc.vector.tensor_tensor(out=umA, in0=tqA, in1=adjA,
                                        op=OP.mult)
                for jt in range(4, NJT):
                    for k in range(2):
                        if (jt, k) not in bset:
                            ts_unit(tqB[:, k, jt - 4, :], jt, h0 + k)
                umB = ump.tile([P, 2, 4, N], bf16, tag="umB", name="umB")
                adjB = adj_all[:, 4:8, :].unsqueeze(1) \
                    .broadcast_to([P, 2, 4, N])
                nc.vector.tensor_tensor(out=umB, in0=tqB, in1=adjB,
                                        op=OP.mult)

                # Pool: evac prev pair's PSUM, then normalize prev pair;
                # out DMA deferred one more pair to keep SP free for
                # the latency-critical transposes
                if pending is not None:
                    ph0, pot, pots = pending
                    for k in range(2):
                        nc.scalar.copy(pots[k][0:AUG, :], pot[k])
                    _fin_transpose_norm(ph0, pots)
                if p >= 2:
                    _out_dma(2 * (p - 2))

                # PE: accumulate jt0-3 then jt4-7
                def mm(jt, start, stop):
                    src = umA if jt < 4 else umB
                    for k in range(2):
                        lhsT = whaug[jt][:, h0 + k, :]
                        for nh in range(2):
                            nc.tensor.matmul(
                                ot[k][:, nh * 512:(nh + 1) * 512], lhsT,
                                src[:, k, jt % 4, nh * 512:(nh + 1) * 512],
                                start=start, stop=stop)

                for jt in range(NJT):
                    mm(jt, start=(jt == 0), stop=(jt == NJT - 1))

                ots_pair = [ots_tiles[(2 * p + k) % 4] for k in range(2)]
                pending = (h0, ot, ots_pair)

            # drain: last pair evac + finalize + remaining out DMAs
            ph0, pot, pots = pending
            for k in range(2):
                nc.scalar.copy(pots[k][0:AUG, :], pot[k])
            _fin_transpose_norm(ph0, pots)
            _out_dma(2)
            _out_dma(4)
            _out_dma(6)

    nc.compile()
    return nc


def _host_prep(h, adj, W, a):
    a1, a2 = a[:, :D_OUT], a[:, D_OUT:]
    w1 = np.einsum("hdf,hf->hd", W, a1).astype(np.float32)
    w2 = np.einsum("hdf,hf->hd", W, a2).astype(np.float32)
    w12 = np.concatenate(
        [w2.T, np.zeros((D_IN, 24), np.float32), w1.T], axis=1).astype(BF16)
    wrs = np.ascontiguousarray(
        W.transpose(1, 0, 2).reshape(D_IN, HF)).astype(BF16)
    in_maps = []
    for b in range(B):
        in_maps.append({
            "hTb": np.ascontiguousarray(h[b].T).astype(BF16),
            "adjT": np.ascontiguousarray(adj[b].T).astype(BF16),
            "wrsb": wrs,
            "w12": w12,
        })
    return in_maps


def kernel(h, adj, W, a):
    from concourse.bass_utils import run_bass_kernel_spmd

    in_maps = _host_prep(np.asarray(h), np.asarray(adj),
                         np.asarray(W), np.asarray(a))
    nc = _build_program()
    res = run_bass_kernel_spmd(nc, in_maps, core_ids=list(range(B)))
    out = np.stack([np.asarray(res.results[b]["out"]) for b in range(B)])
    return out.astype(np.float32)
